# revision 21
# baseline (speedup 1.0000x reference)
"""AtomicComposition histogram kernel for 8 TRN2 NeuronCores.

Reference semantics (nn_AtomicComposition): for each structure (contiguous
256-atom block), count atoms whose atomic number is in ALL_SPECIES =
[1, 6, 7, 8, 16] -> output (32768, 5) float32.

Sharding: data-parallel over structures; each core gets 4096 contiguous
structures (1048576 atoms). The host hands each core its shard TRANSPOSED
([256 atom-slots, 4096 structures], int32) so that on-device the segmented
reduction runs on the TensorEngine:

  - gpsimd casting DMA: int32 DRAM -> bf16 SBUF tiles [128, 1024]
    (two partition groups: atom slots 0-127 / 128-255)
  - VectorE: 5 is_equal compares per tile into a 5-plane mask tile
    [128, 5*1024] (bf16, 4x DVE perf mode)
  - TensorE: ones[128,1]^T @ mask_chunk[128, 512] -> PSUM [1, 512]
    accumulated over the two atom-slot groups; chunks parked at
    32-aligned PSUM partitions
  - ScalarE evacuates PSUM -> SBUF; DMA writes the per-core output
    in species-major layout [5, 4096] f32

The host reassembles/transposes to (32768, 5).
"""

import numpy as np

import concourse.bass as bass
import concourse.mybir as mybir
from concourse.bacc import Bacc
from concourse.tile import TileContext
from concourse.bass_utils import run_bass_kernel_spmd

N_CORES = 8
N_STRUCTURES = 32768
ATOMS_PER = 256
S_LOCAL = N_STRUCTURES // N_CORES          # 4096 structures per core
ATOMS_LOCAL = S_LOCAL * ATOMS_PER          # 1048576 atoms per core
ALL_SPECIES = (1, 6, 7, 8, 16)
N_SPECIES = len(ALL_SPECIES)

P = 128
SBLK = 1024                                # structure columns per block
N_BLK = S_LOCAL // SBLK                    # 4
N_GROUPS = ATOMS_PER // P                  # 2 atom-slot groups


def build_graph(species_vals=ALL_SPECIES):
    nsp = len(species_vals)
    nc = Bacc()

    species = nc.declare_dram_parameter(
        "species_t", [ATOMS_PER, S_LOCAL], mybir.dt.int32, isOutput=False
    )
    # species-major output; host transposes back
    out = nc.declare_dram_parameter(
        "out_t", [nsp, S_LOCAL], mybir.dt.float32, isOutput=True
    )

    with TileContext(nc) as tc:
        with (
            tc.tile_pool(name="const", bufs=1) as const_pool,
            tc.tile_pool(name="sp", bufs=4) as sp_pool,
            tc.tile_pool(name="mask", bufs=4) as mask_pool,
            tc.tile_pool(name="psum", bufs=2, space="PSUM") as psum_pool,
            tc.tile_pool(name="evac", bufs=2) as evac_pool,
        ):
            ones = const_pool.tile([P, 1], mybir.dt.bfloat16)
            nc.vector.memset(ones[:], 1.0)

            # casting loads, tapered columns: two 1024-col pieces (early
            # DVE start) then one 2048-col piece per group
            pieces = [(0, SBLK), (SBLK, SBLK), (2 * SBLK, 2 * SBLK)]
            sp_tiles = {}
            for pi, (col0, w) in enumerate(pieces):
                for g in range(N_GROUPS):
                    tile = sp_pool.tile([P, w], mybir.dt.bfloat16,
                                        tag=f"sp{pi}")
                    nc.gpsimd.dma_start(
                        out=tile[:],
                        in_=species[g * P:(g + 1) * P, col0:col0 + w],
                    )
                    sp_tiles[(g, pi)] = tile

            for pi, (col0, w) in enumerate(pieces):
                masks = []
                for g in range(N_GROUPS):
                    tile = sp_tiles[(g, pi)]
                    mask5 = mask_pool.tile([P, nsp * w], mybir.dt.bfloat16,
                                           tag=f"mask{pi}")
                    for k, z in enumerate(species_vals):
                        nc.vector.tensor_scalar(
                            out=mask5[:, k * w:(k + 1) * w],
                            in0=tile[:],
                            scalar1=float(z),
                            scalar2=None,
                            op0=mybir.AluOpType.is_equal,
                        )
                    masks.append(mask5)

                for ch in range(w // SBLK):
                    c = col0 // SBLK + ch
                    self_block(nc, psum_pool, evac_pool, out, ones, masks,
                               c, ch, plane_w=w)

    nc.finalize()
    return nc


def self_block(nc, psum_pool, evac_pool, out, ones, masks, c, ch,
               plane_w=2 * SBLK):
    """Matmul-reduce one 1024-structure block and DMA its counts out.

    masks: per-group [128, 5*plane_w] bf16 mask tiles (5 species planes
    of plane_w structure columns); this block uses columns
    [ch*1024, (ch+1)*1024) of each plane.
    """
    nsp = len(ALL_SPECIES)
    # merged psum tile [128, 1536] (3 banks): species z<4 at
    # (partition 32z, cols 512*hh); species 4 halves at
    # (partition 0, cols 1024:1536) and (partition 32, cols 1024:1536)
    ps = psum_pool.tile([P, 3 * 512], mybir.dt.float32, tag="ps")
    for m in range(2 * nsp):
        z, hh = divmod(m, 2)
        if z < 4:
            dst = ps[32 * z:32 * z + 1, 512 * hh:512 * (hh + 1)]
            tpos = (0, 32 * z)
        else:
            dst = ps[32 * hh:32 * hh + 1, 1024:1536]
            tpos = (0, 32 * hh)
        col0 = z * plane_w + ch * SBLK + hh * 512
        for g in range(N_GROUPS):
            nc.tensor.matmul(
                out=dst,
                lhsT=ones[:],
                rhs=masks[g][:, col0:col0 + 512],
                start=(g == 0),
                stop=(g == N_GROUPS - 1),
                tile_position=tpos,
            )

    # evacuate the full psum tile -> sbuf in one ScalarE op (cost is
    # free-dim-based, unused partitions are free), then DMA only the
    # meaningful rows (DMA may stride partitions)
    ev = evac_pool.tile([P, 3 * 512], mybir.dt.float32, tag="ev")
    nc.scalar.copy(out=ev[:], in_=ps[:])

    ea = ev[:, 0:SBLK].rearrange("(zz r) q -> zz r q", zz=4, r=32)[:, 0]
    nc.sync.dma_start(out=out[0:4, c * SBLK:(c + 1) * SBLK], in_=ea)
    e4 = ev[:, 1024:1536].rearrange("(zz r) q -> zz r q", zz=4, r=32)[0:2, 0]
    nc.sync.dma_start(out=out[4:5, c * SBLK:(c + 1) * SBLK], in_=e4)


_GRAPH_CACHE = {}


def _get_graph(species_vals):
    key = tuple(int(v) for v in species_vals)
    if key not in _GRAPH_CACHE:
        _GRAPH_CACHE[key] = build_graph(key)
    return _GRAPH_CACHE[key]


def make_in_maps(species: np.ndarray) -> list:
    # shard by contiguous structure blocks; transpose each shard to
    # [ATOMS_PER, S_LOCAL] so each core's partition dim is the atom slot
    shards = species.reshape(N_CORES, S_LOCAL, ATOMS_PER)
    return [
        {"species_t": np.ascontiguousarray(shards[i].T)} for i in range(N_CORES)
    ]


def kernel(**inputs) -> np.ndarray:
    species = np.asarray(inputs["species"], dtype=np.int32)
    all_species = np.asarray(inputs["all_species"]).reshape(-1)
    assert species.shape == (N_STRUCTURES * ATOMS_PER,), species.shape

    nc = _get_graph(all_species)

    in_maps = make_in_maps(species)
    res = run_bass_kernel_spmd(nc, in_maps, core_ids=list(range(N_CORES)))
    # per-core out_t is [n_species, S_LOCAL]; reassemble to (N_STRUCTURES, n)
    outs = [np.asarray(res.results[i]["out_t"]).T for i in range(N_CORES)]
    return np.ascontiguousarray(
        np.concatenate(outs, axis=0), dtype=np.float32
    )
